# revision 20
# baseline (speedup 1.0000x reference)
"""Causal self-attention (T=4096, C=2048, 16 heads) on 8 TRN2 NeuronCores.

Sharding: tensor-parallel over heads (2 heads/core) for QKV + attention,
then per-head AllToAlls redistribute the attention output to
token-parallel (512 tokens/core) for the output projection. No reduction
collective is needed: each core computes full output rows for its token
slice and the host concatenates.

All matmuls run bf16 (inputs converted to bf16 on the host, halving DMA
bytes; PSUM accumulation stays fp32). Scores are computed transposed
(keys on partitions, queries free) so P@V needs no transposes. Softmax
is normalized on the SENDER side before the AllToAll: denominators come
from pair-summed e-tiles through a half-rate ones-matmul, reciprocal on
DVE, partition-broadcast on GpSimd, and one DVE multiply that also does
the PSUM->SBUF bf16 conversion. The A2A then carries ready-to-use
attention rows and phase 3 is pure matmul. Causal masking uses
affine_select directly on the exp tiles (GpSimd), with the diagonal
k-tiles processed first in each chunk so their longer dependency chain
hides under the remaining tiles. Exp runs on [128,1024] PSUM score
groups to amortize the activation engine's fixed overhead.
"""
import sys
import types

sys.path.insert(0, "/opt/trn_rl_repo")

import ml_dtypes
import numpy as np

from concourse import bacc, tile
import concourse.mybir as mybir
from concourse.bass_utils import run_bass_kernel_spmd

F32 = mybir.dt.float32
BF16 = mybir.dt.bfloat16
NP_BF16 = np.dtype(ml_dtypes.bfloat16)

T, C = 4096, 2048
H, D = 16, 128
W = 8                  # cores
HL = H // W            # heads per core (2)
CL = HL * D            # local attention-output columns (256)
KT = C // 128          # contraction tiles (16)
TC1 = 512              # phase-1 token chunk
NC1 = T // TC1         # 8
TC2 = 512              # phase-2/3 token chunk
NC2 = T // TC2         # 8
TL = T // W            # tokens per core for the projection (512)
SCALE = float(1.0 / np.sqrt(D))

TRACE = False          # test harness sets kernel.TRACE = True for profiling
LAST_RESULT = {}       # test harness reads exec_time_ns from here

_cache = {}


def _build():
    nc = bacc.Bacc("TRN2", target_bir_lowering=False, debug=False, num_devices=W)
    xT_d = nc.dram_tensor("xT", [C, T], BF16, kind="ExternalInput")
    wqkT_d = nc.dram_tensor("wqkT", [C, 2 * CL], BF16, kind="ExternalInput")
    wvT_d = nc.dram_tensor("wvT", [C, CL], BF16, kind="ExternalInput")
    wpT_d = nc.dram_tensor("wpT", [C, C], BF16, kind="ExternalInput")
    out_d = nc.dram_tensor("out", [TL, C], F32, kind="ExternalOutput")

    with tile.TileContext(nc) as tc:
        with tc.tile_pool(name="res", bufs=1) as res, \
             tc.tile_pool(name="dram", bufs=1, space="DRAM") as dram:
            # per-head A2A buffers (bf16, already normalized): shard j = my
            # token chunk j of my 128 head-columns.
            a2a_in = [dram.tile([W, 128, TC2], BF16, tag=f"a2a_in{h}",
                                name=f"a2a_in{h}") for h in range(HL)]
            a2a_out = [dram.tile([W, 128, TC2], BF16, tag=f"a2a_out{h}",
                                 name=f"a2a_out{h}") for h in range(HL)]

            # resident q/k (transposed, [d, t]) and V ([s, d]), all bf16
            qT = [res.tile([128, T], BF16, tag=f"qT{h}", name=f"qT{h}")
                  for h in range(HL)]
            kT = [res.tile([128, T], BF16, tag=f"kT{h}", name=f"kT{h}")
                  for h in range(HL)]
            V = [res.tile([128, CL], BF16, tag=f"V{i}", name=f"V{i}")
                 for i in range(T // 128)]

            ones32 = res.tile([128, 1], F32, tag="ones32")
            nc.gpsimd.memset(ones32[:], 1.0)
            ones = res.tile([128, 1], BF16, tag="ones")
            nc.vector.tensor_copy(ones[:], ones32[:])
            # warm the Exp table set during the startup DMA wait (the
            # ~2.7us ACT_TABLE_LOAD otherwise lands at phase-2 start)
            expwarm = res.tile([1, 1], F32, tag="expwarm")
            nc.scalar.activation(expwarm[:], ones32[0:1, 0:1],
                                 mybir.ActivationFunctionType.Exp)

            # ---------------- phase 1: QKV projection (bf16) ----------------
            with tc.tile_pool(name="wpool", bufs=1) as wpool, \
                 tc.tile_pool(name="xpool", bufs=2) as xpool, \
                 tc.tile_pool(name="ps1", bufs=3, space="PSUM") as ps1:
                # one DMA per [C, *] block: each trigger costs ~0.6us of
                # queue time, so merge the 16 contraction tiles per load
                def load_x_chunk(j, split=1):
                    t_ = xpool.tile([128, KT, TC1], BF16, tag="x",
                                    name=f"x{j}")
                    ks = KT // split
                    for a in range(split):
                        nc.sync.dma_start(
                            t_[:, a * ks:(a + 1) * ks, :],
                            xT_d.ap()[a * ks * 128:(a + 1) * ks * 128,
                                      j * TC1:(j + 1) * TC1].rearrange(
                                "(k p) t -> p k t", p=128),
                        )
                    return t_

                wqk = []
                xt0 = None
                for m in range(4):
                    t_ = wpool.tile([128, KT, 128], BF16,
                                    tag=f"wqk{m}", name=f"wqk{m}")
                    nc.sync.dma_start(
                        t_[:],
                        wqkT_d.ap()[:, m * 128:(m + 1) * 128].rearrange(
                            "(k p) t -> p k t", p=128),
                    )
                    wqk.append(t_)
                    if m == 0:
                        # x chunk 0 right after the first weight block so
                        # the first accumulation group starts ASAP
                        xt0 = load_x_chunk(0, split=4)
                wv = wpool.tile([128, KT, CL], BF16, tag="wv", name="wv")
                nc.sync.dma_start(
                    wv[:], wvT_d.ap().rearrange("(k p) t -> p k t", p=128))

                wp = []
                for j in range(NC1):
                    xt = xt0 if j == 0 else load_x_chunk(j)
                    if j == 5:
                        # prefetch the projection weight mid-phase-1: the
                        # sync queue has slack here, and is then free for
                        # the phase-2 attention stores / broadcasts
                        for oc in range(C // 512):
                            t_ = res.tile([128, KT, 512], BF16,
                                          tag=f"wp{oc}", name=f"wp{oc}")
                            nc.sync.dma_start(
                                t_[:],
                                wpT_d.ap()[:, oc * 512:(oc + 1) * 512]
                                .rearrange("(k p) t -> p k t", p=128),
                            )
                            wp.append(t_)
                    # qT/kT for both heads: out[d, t] accumulated over c
                    for m in range(4):
                        pq = ps1.tile([128, TC1], F32, tag="pqk")
                        for k in range(KT):
                            nc.tensor.matmul(pq[:], wqk[m][:, k, :],
                                             xt[:, k, :],
                                             start=(k == 0), stop=(k == KT - 1))
                        dest = qT[m] if m < HL else kT[m - HL]
                        nc.vector.tensor_copy(
                            dest[:, j * TC1:(j + 1) * TC1], pq[:])
                    # V: out[t, d] accumulated over c
                    for tt in range(TC1 // 128):
                        pv = ps1.tile([128, CL], F32, tag="pv")
                        for k in range(KT):
                            nc.tensor.matmul(
                                pv[:],
                                xt[:, k, tt * 128:(tt + 1) * 128],
                                wv[:, k, :],
                                start=(k == 0), stop=(k == KT - 1))
                        nc.scalar.copy(V[j * (TC1 // 128) + tt][:], pv[:])

            # ---------------- phases 2+3 pools ----------------
            with tc.tile_pool(name="p2e", bufs=4) as p2e, \
                 tc.tile_pool(name="p2p", bufs=3) as p2p, \
                 tc.tile_pool(name="p2a", bufs=3) as p2a, \
                 tc.tile_pool(name="p2n", bufs=2) as p2n, \
                 tc.tile_pool(name="p3a", bufs=1) as p3a, \
                 tc.tile_pool(name="p3o", bufs=2) as p3o:
                attn = [None] * KT

                def load_attn(h):
                    # one 1MB DMA for the head's whole A2A result
                    t_ = p3a.tile([128, W, TL], BF16, tag=f"at{h}",
                                  name=f"at{h}")
                    nc.sync.dma_start(
                        t_[:],
                        a2a_out[h][:, :, :].rearrange("i p t -> p i t"))
                    for i in range(W):
                        attn[i * HL + h] = t_[:, i, :]

                # ---------------- phase 2: attention (bf16) ----------------
                with tc.tile_pool(name="ps2s", bufs=2, space="PSUM") as ps2s, \
                     tc.tile_pool(name="ps2o", bufs=2, space="PSUM") as ps2o, \
                     tc.tile_pool(name="ps2d", bufs=2, space="PSUM") as ps2d:
                    for h in range(HL):
                        pending = [None]

                        def flush_norm(h=h, pending=pending):
                            # softmax-normalize chunk j's P@V on the sender.
                            # DVE only touches PSUM (copy + reciprocal, no
                            # cross-engine waits); the broadcast and multiply
                            # both run on GpSimd so a slow hop never blocks
                            # the DVE queue that feeds the tensor engine.
                            j, po_t, pd_t = pending[0]
                            pending[0] = None
                            att = p2a.tile([128, TC2], BF16, tag="att",
                                           name=f"att{h}_{j}")
                            nc.vector.tensor_copy(att[:], po_t[:])
                            rec = p2n.tile([1, TC2], F32, tag="rec",
                                           name=f"rec{h}_{j}")
                            nc.vector.reciprocal_approx_fast(
                                rec[:], pd_t[0:1, :])
                            r128 = p2n.tile([128, TC2], F32, tag="r128",
                                            name=f"r128_{h}_{j}")
                            nc.gpsimd.partition_broadcast(r128[:], rec[:])
                            nc.gpsimd.tensor_mul(att[:], att[:], r128[:])
                            nc.sync.dma_start(a2a_in[h][j, :, :], att[:])

                        # big chunks first: deep pipelines from the start,
                        # and the small chunks' serial chains at the end
                        # overlap the next head / phase 3
                        for j in reversed(range(NC2)):
                            # natural k order: the diagonal tiles' longer
                            # mask chain lands at the chunk end where the
                            # next chunk's score matmuls cover it
                            ks = list(range(0, 4 * (j + 1)))
                            G = len(ks) // 2
                            po = ps2o.tile([128, TC2], F32, tag="po",
                                           name=f"po{h}_{j}")
                            pd = ps2d.tile([128, TC2], F32, tag="pd",
                                           name=f"pd{h}_{j}")
                            prev_pair = None   # for quad-summed bulk pd
                            pd_started = [False]

                            def pd_mm(src, last, pd=pd, pd_started=pd_started):
                                nc.tensor.matmul(pd[0:1, :], ones[:], src[:],
                                                 start=not pd_started[0],
                                                 stop=last)
                                pd_started[0] = True

                            for g in range(G):
                                k0, k1 = ks[2 * g], ks[2 * g + 1]
                                diag = g >= G - 2
                                # diagonal tile dk keeps only t >= 128*dk;
                                # skip the all-masked column block entirely
                                off0 = 128 * (k0 - 4 * j) if diag else 0
                                off1 = 128 * (k1 - 4 * j) if diag else 0
                                sc = ps2s.tile([128, 2 * TC2], F32, tag="sc",
                                               name=f"sc{h}_{j}_{g}")
                                for half, kk, off in ((0, k0, off0),
                                                      (1, k1, off1)):
                                    nc.tensor.matmul(
                                        sc[:, half * TC2 + off:
                                           (half + 1) * TC2],
                                        kT[h][:, kk * 128:(kk + 1) * 128],
                                        qT[h][:, j * TC2 + off:
                                              (j + 1) * TC2],
                                        start=True, stop=True)
                                eg = p2e.tile([128, 2 * TC2], BF16, tag="eg",
                                              name=f"eg{h}_{j}_{g}")
                                nc.scalar.activation(
                                    eg[:], sc[:],
                                    mybir.ActivationFunctionType.Exp,
                                    scale=SCALE)
                                if diag:
                                    # zero s > t in place (also clears the
                                    # skipped-region garbage, which lies
                                    # below the diagonal band)
                                    for half, kk in ((0, k0), (1, k1)):
                                        dk = kk - 4 * j
                                        nc.gpsimd.affine_select(
                                            out=eg[:, half * TC2:
                                                   (half + 1) * TC2],
                                            in_=eg[:, half * TC2:
                                                  (half + 1) * TC2],
                                            compare_op=mybir.AluOpType.is_ge,
                                            fill=0.0,
                                            base=-128 * dk,
                                            channel_multiplier=-1,
                                            pattern=[[1, TC2]],
                                        )
                                pair = p2p.tile([128, TC2], BF16, tag="pair",
                                                name=f"pair{h}_{j}_{g}")
                                nc.vector.tensor_add(
                                    pair[:], eg[:, 0:TC2], eg[:, TC2:2 * TC2])
                                if g == 0 and pending[0] is not None:
                                    flush_norm()
                                if diag:
                                    pd_mm(pair, g == G - 1)
                                elif prev_pair is not None:
                                    # quad: one ones-matmul per two bulk pairs
                                    quad = p2p.tile([128, TC2], BF16,
                                                    tag="quad",
                                                    name=f"quad{h}_{j}_{g}")
                                    nc.vector.tensor_add(
                                        quad[:], prev_pair[:], pair[:])
                                    pd_mm(quad, False)
                                    prev_pair = None
                                else:
                                    prev_pair = pair
                                for half, kk, off in ((0, k0, off0),
                                                      (1, k1, off1)):
                                    nc.tensor.matmul(
                                        po[:, off:TC2],
                                        V[kk][:, h * 128:(h + 1) * 128],
                                        eg[:, half * TC2 + off:
                                           (half + 1) * TC2],
                                        start=(g == 0 and half == 0),
                                        stop=(g == G - 1 and half == 1))
                            pending[0] = (j, po, pd)
                        flush_norm()
                        # fire this head's A2A as soon as its chunks are
                        # written; head 0's collective overlaps head 1
                        nc.gpsimd.collective_compute(
                            "AllToAll",
                            mybir.AluOpType.bypass,
                            ins=[a2a_in[h].opt()],
                            outs=[a2a_out[h].opt()],
                            replica_groups=[list(range(W))],
                        )
                        if h == 0:
                            # head-0 attn load: waits on head-0's A2A and
                            # transfers during head-1 compute
                            load_attn(0)

                # ---------------- phase 3: output projection (bf16) ----------------
                load_attn(1)
                # even kc first (head-0 sourced, available before the second
                # A2A): ALL 128 even matmuls run as A2A cover, spilling the
                # partial sums to SBUF so the 8 PSUM banks can be reused;
                # the odd accumulation then adds the spill back via DVE
                with tc.tile_pool(name="ps3", bufs=1, space="PSUM") as ps3, \
                     tc.tile_pool(name="p3e", bufs=1) as p3e:
                    evn = {}
                    for og in range(2):
                        for oc in (2 * og, 2 * og + 1):
                            for tt in range(TL // 128):
                                po3 = ps3.tile([128, 512], F32,
                                               tag=f"po3_{oc % 2}_{tt}",
                                               name=f"po3e_{oc}_{tt}")
                                for kc in range(0, KT, 2):
                                    nc.tensor.matmul(
                                        po3[:],
                                        attn[kc][:, tt * 128:(tt + 1) * 128],
                                        wp[oc][:, kc, :],
                                        start=(kc == 0), stop=(kc == KT - 2))
                                ev = p3e.tile([128, 512], F32,
                                              tag=f"ev{oc}_{tt}",
                                              name=f"ev{oc}_{tt}")
                                nc.scalar.copy(ev[:], po3[:])
                                evn[(oc, tt)] = ev
                    for og in range(2):
                        for oc in (2 * og, 2 * og + 1):
                            for tt in range(TL // 128):
                                po3 = ps3.tile([128, 512], F32,
                                               tag=f"po3_{oc % 2}_{tt}",
                                               name=f"po3o_{oc}_{tt}")
                                for kc in range(1, KT, 2):
                                    nc.tensor.matmul(
                                        po3[:],
                                        attn[kc][:, tt * 128:(tt + 1) * 128],
                                        wp[oc][:, kc, :],
                                        start=(kc == 1), stop=(kc == KT - 1))
                                ob = p3o.tile([128, 512], F32, tag="ob")
                                nc.vector.tensor_add(
                                    ob[:], po3[:], evn[(oc, tt)][:])
                                nc.sync.dma_start(
                                    out_d.ap()[tt * 128:(tt + 1) * 128,
                                               oc * 512:(oc + 1) * 512],
                                    ob[:])

    nc.compile()
    return nc


def _maybe_install_trace_hook():
    try:
        import antenv
        from trn_agent_boot.trn_boot import _ntff_profile_via_ctypes
        hook = _ntff_profile_via_ctypes("/opt/axon/libaxon_pjrt.so")
        mod = types.ModuleType("antenv.axon_hooks")
        mod.get_axon_ntff_profile_hook = lambda: hook
        mod.set_axon_ntff_profile_hook = lambda h: None
        sys.modules["antenv.axon_hooks"] = mod
        antenv.axon_hooks = mod
        return True
    except Exception:
        return False


def kernel(x, w_attn, w_proj):
    x = np.ascontiguousarray(x, dtype=np.float32)
    w_attn = np.ascontiguousarray(w_attn, dtype=np.float32)
    w_proj = np.ascontiguousarray(w_proj, dtype=np.float32)

    if "nc" not in _cache:
        _cache["nc"] = _build()
    nc = _cache["nc"]

    xT = np.ascontiguousarray(x.T).astype(NP_BF16)
    wpT = np.ascontiguousarray(w_proj.T).astype(NP_BF16)
    in_maps = []
    for c in range(W):
        r0 = CL * c
        wqk = np.concatenate(
            [w_attn[r0:r0 + CL], w_attn[C + r0:C + r0 + CL]], axis=0)
        wqkT = np.ascontiguousarray(wqk.T).astype(NP_BF16)
        wvT = np.ascontiguousarray(
            w_attn[2 * C + r0:2 * C + r0 + CL].T).astype(NP_BF16)
        in_maps.append({"xT": xT, "wqkT": wqkT, "wvT": wvT, "wpT": wpT})

    trace = TRACE and _maybe_install_trace_hook()
    res = run_bass_kernel_spmd(nc, in_maps, list(range(W)), trace=trace)
    LAST_RESULT["exec_time_ns"] = res.exec_time_ns

    return np.concatenate([res.results[c]["out"] for c in range(W)], axis=0)


# revision 26
# speedup vs baseline: 1.0789x; 1.0789x over previous
"""Causal self-attention (T=4096, C=2048, 16 heads) on 8 TRN2 NeuronCores.

Sharding: tensor-parallel over heads (2 heads/core) for QKV + attention,
then per-head AllToAlls redistribute the attention output to
token-parallel (512 tokens/core) for the output projection. No reduction
collective is needed: each core computes full output rows for its token
slice and the host concatenates.

All matmuls run bf16 (inputs converted to bf16 on the host, halving DMA
bytes; PSUM accumulation stays fp32). Scores are computed transposed
(keys on partitions, queries free) so P@V needs no transposes. Softmax
is normalized on the SENDER side before the AllToAll: denominators come
from pair-summed e-tiles through a half-rate ones-matmul, reciprocal on
DVE, partition-broadcast on GpSimd, and one DVE multiply that also does
the PSUM->SBUF bf16 conversion. The A2A then carries ready-to-use
attention rows and phase 3 is pure matmul. Causal masking uses
affine_select directly on the exp tiles (GpSimd), with the diagonal
k-tiles processed first in each chunk so their longer dependency chain
hides under the remaining tiles. Exp runs on [128,1024] PSUM score
groups to amortize the activation engine's fixed overhead.
"""
import sys
import types

sys.path.insert(0, "/opt/trn_rl_repo")

import ml_dtypes
import numpy as np

from concourse import bacc, tile
import concourse.mybir as mybir
from concourse.bass_utils import run_bass_kernel_spmd

F32 = mybir.dt.float32
BF16 = mybir.dt.bfloat16
NP_BF16 = np.dtype(ml_dtypes.bfloat16)

T, C = 4096, 2048
H, D = 16, 128
W = 8                  # cores
HL = H // W            # heads per core (2)
CL = HL * D            # local attention-output columns (256)
KT = C // 128          # contraction tiles (16)
TC1 = 512              # phase-1 token chunk
NC1 = T // TC1         # 8
TC2 = 512              # phase-2/3 token chunk
NC2 = T // TC2         # 8
TL = T // W            # tokens per core for the projection (512)
SCALE = float(1.0 / np.sqrt(D))

TRACE = False          # test harness sets kernel.TRACE = True for profiling
LAST_RESULT = {}       # test harness reads exec_time_ns from here

_cache = {}


def _build():
    nc = bacc.Bacc("TRN2", target_bir_lowering=False, debug=False, num_devices=W)
    xT_d = nc.dram_tensor("xT", [C, T], BF16, kind="ExternalInput")
    wqkT_d = nc.dram_tensor("wqkT", [C, 2 * CL], BF16, kind="ExternalInput")
    wvT_d = nc.dram_tensor("wvT", [C, CL], BF16, kind="ExternalInput")
    wpT_d = nc.dram_tensor("wpT", [C, C], BF16, kind="ExternalInput")
    out_d = nc.dram_tensor("out", [TL, C], F32, kind="ExternalOutput")

    with tile.TileContext(nc) as tc:
        with tc.tile_pool(name="res", bufs=1) as res, \
             tc.tile_pool(name="dram", bufs=1, space="DRAM") as dram:
            # per-head A2A buffers (bf16, already normalized): shard j = my
            # token chunk j of my 128 head-columns.
            a2a_in = [dram.tile([W, 128, TC2], BF16, tag=f"a2a_in{h}",
                                name=f"a2a_in{h}") for h in range(HL)]
            a2a_out = [dram.tile([W, 128, TC2], BF16, tag=f"a2a_out{h}",
                                 name=f"a2a_out{h}") for h in range(HL)]

            # resident q/k (transposed, [d, t]) and V ([s, d]), all bf16
            qT = [res.tile([128, T], BF16, tag=f"qT{h}", name=f"qT{h}")
                  for h in range(HL)]
            kT = [res.tile([128, T], BF16, tag=f"kT{h}", name=f"kT{h}")
                  for h in range(HL)]
            V = [res.tile([128, CL], BF16, tag=f"V{i}", name=f"V{i}")
                 for i in range(T // 128)]

            ones32 = res.tile([128, 1], F32, tag="ones32")
            nc.gpsimd.memset(ones32[:], 1.0)
            ones = res.tile([128, 1], BF16, tag="ones")
            nc.vector.tensor_copy(ones[:], ones32[:])
            # warm the Exp table set during the startup DMA wait (the
            # ~2.7us ACT_TABLE_LOAD otherwise lands at phase-2 start)
            expwarm = res.tile([1, 1], F32, tag="expwarm")
            nc.scalar.activation(expwarm[:], ones32[0:1, 0:1],
                                 mybir.ActivationFunctionType.Exp)

            # ---------------- phase 1: QKV projection (bf16) ----------------
            with tc.tile_pool(name="wpool", bufs=1) as wpool, \
                 tc.tile_pool(name="xpool", bufs=3) as xpool, \
                 tc.tile_pool(name="ps1", bufs=3, space="PSUM") as ps1:
                # one DMA per [C, *] block: each trigger costs ~0.6us of
                # queue time, so merge the 16 contraction tiles per load
                def load_x_chunk(j, split=1):
                    t_ = xpool.tile([128, KT, TC1], BF16, tag="x",
                                    name=f"x{j}")
                    ks = KT // split
                    for a in range(split):
                        nc.sync.dma_start(
                            t_[:, a * ks:(a + 1) * ks, :],
                            xT_d.ap()[a * ks * 128:(a + 1) * ks * 128,
                                      j * TC1:(j + 1) * TC1].rearrange(
                                "(k p) t -> p k t", p=128),
                        )
                    return t_

                wqk = []
                xt0 = None
                for m in range(4):
                    t_ = wpool.tile([128, KT, 128], BF16,
                                    tag=f"wqk{m}", name=f"wqk{m}")
                    nc.sync.dma_start(
                        t_[:],
                        wqkT_d.ap()[:, m * 128:(m + 1) * 128].rearrange(
                            "(k p) t -> p k t", p=128),
                    )
                    wqk.append(t_)
                    if m == 0:
                        # x chunk 0 right after the first weight block so
                        # the first accumulation group starts ASAP
                        xt0 = load_x_chunk(0, split=4)
                wv = wpool.tile([128, KT, CL], BF16, tag="wv", name="wv")
                nc.sync.dma_start(
                    wv[:], wvT_d.ap().rearrange("(k p) t -> p k t", p=128))

                wp = []
                xts = {0: xt0, 1: load_x_chunk(1), 2: load_x_chunk(2)}
                for j in range(NC1):
                    xt = xts.pop(j)
                    # depth-2 prefetch (xpool bufs=3) keeps the x stream
                    # ahead of the wp transfer inserted below
                    if j + 2 < NC1:
                        xts[j + 2] = load_x_chunk(j + 2)
                    if j == 1:
                        # prefetch the projection weight early in phase 1:
                        # the sync queue is then free well before the
                        # phase-2 attention stores / broadcasts need it
                        for oc in range(C // 512):
                            t_ = res.tile([128, KT, 512], BF16,
                                          tag=f"wp{oc}", name=f"wp{oc}")
                            nc.sync.dma_start(
                                t_[:],
                                wpT_d.ap()[:, oc * 512:(oc + 1) * 512]
                                .rearrange("(k p) t -> p k t", p=128),
                            )
                            wp.append(t_)
                    # qT/kT for both heads: out[d, t] accumulated over c
                    for m in range(4):
                        pq = ps1.tile([128, TC1], F32, tag="pqk")
                        for k in range(KT):
                            nc.tensor.matmul(pq[:], wqk[m][:, k, :],
                                             xt[:, k, :],
                                             start=(k == 0), stop=(k == KT - 1))
                        dest = qT[m] if m < HL else kT[m - HL]
                        nc.vector.tensor_copy(
                            dest[:, j * TC1:(j + 1) * TC1], pq[:])
                    # V: out[t, d] accumulated over c
                    for tt in range(TC1 // 128):
                        pv = ps1.tile([128, CL], F32, tag="pv")
                        for k in range(KT):
                            nc.tensor.matmul(
                                pv[:],
                                xt[:, k, tt * 128:(tt + 1) * 128],
                                wv[:, k, :],
                                start=(k == 0), stop=(k == KT - 1))
                        nc.scalar.copy(V[j * (TC1 // 128) + tt][:], pv[:])

            # ---------------- phases 2+3 pools ----------------
            with tc.tile_pool(name="p2e", bufs=4) as p2e, \
                 tc.tile_pool(name="p2p", bufs=3) as p2p, \
                 tc.tile_pool(name="p2a", bufs=3) as p2a, \
                 tc.tile_pool(name="p2n", bufs=2) as p2n, \
                 tc.tile_pool(name="p3a", bufs=1) as p3a, \
                 tc.tile_pool(name="p3o", bufs=2) as p3o:
                attn = [None] * KT

                def load_attn(h):
                    # one 1MB DMA for the head's whole A2A result
                    t_ = p3a.tile([128, W, TL], BF16, tag=f"at{h}",
                                  name=f"at{h}")
                    nc.sync.dma_start(
                        t_[:],
                        a2a_out[h][:, :, :].rearrange("i p t -> p i t"))
                    for i in range(W):
                        attn[i * HL + h] = t_[:, i, :]

                # ---------------- phase 2: attention (bf16) ----------------
                with tc.tile_pool(name="ps2s", bufs=2, space="PSUM") as ps2s, \
                     tc.tile_pool(name="ps2o", bufs=2, space="PSUM") as ps2o, \
                     tc.tile_pool(name="ps2d", bufs=2, space="PSUM") as ps2d:
                    for h in range(HL):
                        pending = [None]
                        inflight = [None]

                        def flush_a(h=h, pending=pending,
                                    inflight=inflight):
                            # stage a: reciprocal (DVE) + GpSimd partition
                            # broadcast; stage b only runs a group later,
                            # so the broadcast has a full group of slack
                            # before the DVE multiply needs it
                            j, po_t, pd_t = pending[0]
                            pending[0] = None
                            rec = p2n.tile([1, TC2], F32, tag="rec",
                                           name=f"rec{h}_{j}")
                            nc.vector.reciprocal_approx_fast(
                                rec[:], pd_t[0:1, :])
                            r128 = p2n.tile([128, TC2], F32, tag="r128",
                                            name=f"r128_{h}_{j}")
                            nc.gpsimd.partition_broadcast(r128[:], rec[:])
                            inflight[0] = (j, po_t, r128)

                        def flush_b(h=h, inflight=inflight):
                            # stage b (one group later, broadcast done):
                            # one DVE multiply normalizes and converts
                            # PSUM f32 -> SBUF bf16, then store
                            j, po_t, r128 = inflight[0]
                            inflight[0] = None
                            att = p2a.tile([128, TC2], BF16, tag="att",
                                           name=f"att{h}_{j}")
                            nc.vector.tensor_mul(att[:], po_t[:], r128[:])
                            nc.sync.dma_start(a2a_in[h][j, :, :], att[:])

                        # big chunks first: deep pipelines from the start,
                        # and the small chunks' serial chains at the end
                        # overlap the next head / phase 3
                        for j in reversed(range(NC2)):
                            # natural k order: the diagonal tiles' longer
                            # mask chain lands at the chunk end where the
                            # next chunk's score matmuls cover it
                            ks = list(range(0, 4 * (j + 1)))
                            G = len(ks) // 2
                            po = ps2o.tile([128, TC2], F32, tag="po",
                                           name=f"po{h}_{j}")
                            pd = ps2d.tile([128, TC2], F32, tag="pd",
                                           name=f"pd{h}_{j}")
                            prev_pair = None   # for quad-summed bulk pd
                            pd_started = [False]

                            def pd_mm(src, last, pd=pd, pd_started=pd_started):
                                nc.tensor.matmul(pd[0:1, :], ones[:], src[:],
                                                 start=not pd_started[0],
                                                 stop=last)
                                pd_started[0] = True

                            for g in range(G):
                                k0, k1 = ks[2 * g], ks[2 * g + 1]
                                diag = g >= G - 2
                                # diagonal tile dk keeps only t >= 128*dk;
                                # skip the all-masked column block entirely
                                off0 = 128 * (k0 - 4 * j) if diag else 0
                                off1 = 128 * (k1 - 4 * j) if diag else 0
                                sc = ps2s.tile([128, 2 * TC2], F32, tag="sc",
                                               name=f"sc{h}_{j}_{g}")
                                for half, kk, off in ((0, k0, off0),
                                                      (1, k1, off1)):
                                    nc.tensor.matmul(
                                        sc[:, half * TC2 + off:
                                           (half + 1) * TC2],
                                        kT[h][:, kk * 128:(kk + 1) * 128],
                                        qT[h][:, j * TC2 + off:
                                              (j + 1) * TC2],
                                        start=True, stop=True)
                                eg = p2e.tile([128, 2 * TC2], BF16, tag="eg",
                                              name=f"eg{h}_{j}_{g}")
                                nc.scalar.activation(
                                    eg[:], sc[:],
                                    mybir.ActivationFunctionType.Exp,
                                    scale=SCALE)
                                if diag:
                                    # zero s > t in place (also clears the
                                    # skipped-region garbage, which lies
                                    # below the diagonal band)
                                    for half, kk in ((0, k0), (1, k1)):
                                        dk = kk - 4 * j
                                        nc.gpsimd.affine_select(
                                            out=eg[:, half * TC2:
                                                   (half + 1) * TC2],
                                            in_=eg[:, half * TC2:
                                                  (half + 1) * TC2],
                                            compare_op=mybir.AluOpType.is_ge,
                                            fill=0.0,
                                            base=-128 * dk,
                                            channel_multiplier=-1,
                                            pattern=[[1, TC2]],
                                        )
                                pair = p2p.tile([128, TC2], BF16, tag="pair",
                                                name=f"pair{h}_{j}_{g}")
                                nc.vector.tensor_add(
                                    pair[:], eg[:, 0:TC2], eg[:, TC2:2 * TC2])
                                if g == 0 and pending[0] is not None:
                                    flush_a()
                                elif g == 1 and inflight[0] is not None:
                                    flush_b()
                                if diag:
                                    pd_mm(pair, g == G - 1)
                                elif prev_pair is not None:
                                    # quad: one ones-matmul per two bulk pairs
                                    quad = p2p.tile([128, TC2], BF16,
                                                    tag="quad",
                                                    name=f"quad{h}_{j}_{g}")
                                    nc.vector.tensor_add(
                                        quad[:], prev_pair[:], pair[:])
                                    pd_mm(quad, False)
                                    prev_pair = None
                                else:
                                    prev_pair = pair
                                for half, kk, off in ((0, k0, off0),
                                                      (1, k1, off1)):
                                    nc.tensor.matmul(
                                        po[:, off:TC2],
                                        V[kk][:, h * 128:(h + 1) * 128],
                                        eg[:, half * TC2 + off:
                                           (half + 1) * TC2],
                                        start=(g == 0 and half == 0),
                                        stop=(g == G - 1 and half == 1))
                            pending[0] = (j, po, pd)
                        flush_a()
                        flush_b()
                        # fire this head's A2A as soon as its chunks are
                        # written; head 0's collective overlaps head 1
                        nc.gpsimd.collective_compute(
                            "AllToAll",
                            mybir.AluOpType.bypass,
                            ins=[a2a_in[h].opt()],
                            outs=[a2a_out[h].opt()],
                            replica_groups=[list(range(W))],
                        )
                        if h == 0:
                            # head-0 attn load: waits on head-0's A2A and
                            # transfers during head-1 compute
                            load_attn(0)

                # ---------------- phase 3: output projection (bf16) ----------------
                load_attn(1)
                # even kc first (head-0 sourced, available before the second
                # A2A): ALL 128 even matmuls run as A2A cover, spilling the
                # partial sums to SBUF so the 8 PSUM banks can be reused;
                # the odd accumulation then adds the spill back via DVE
                with tc.tile_pool(name="ps3", bufs=1, space="PSUM") as ps3, \
                     tc.tile_pool(name="p3e", bufs=1) as p3e:
                    evn = {}
                    for og in range(2):
                        for oc in (2 * og, 2 * og + 1):
                            for tt in range(TL // 128):
                                po3 = ps3.tile([128, 512], F32,
                                               tag=f"po3_{oc % 2}_{tt}",
                                               name=f"po3e_{oc}_{tt}")
                                for kc in range(0, KT, 2):
                                    nc.tensor.matmul(
                                        po3[:],
                                        attn[kc][:, tt * 128:(tt + 1) * 128],
                                        wp[oc][:, kc, :],
                                        start=(kc == 0), stop=(kc == KT - 2))
                                ev = p3e.tile([128, 512], F32,
                                              tag=f"ev{oc}_{tt}",
                                              name=f"ev{oc}_{tt}")
                                nc.scalar.copy(ev[:], po3[:])
                                evn[(oc, tt)] = ev
                    for og in range(2):
                        for oc in (2 * og, 2 * og + 1):
                            for tt in range(TL // 128):
                                po3 = ps3.tile([128, 512], F32,
                                               tag=f"po3_{oc % 2}_{tt}",
                                               name=f"po3o_{oc}_{tt}")
                                for kc in range(1, KT, 2):
                                    nc.tensor.matmul(
                                        po3[:],
                                        attn[kc][:, tt * 128:(tt + 1) * 128],
                                        wp[oc][:, kc, :],
                                        start=(kc == 1), stop=(kc == KT - 1))
                                ob = p3o.tile([128, 512], F32, tag="ob")
                                nc.vector.tensor_add(
                                    ob[:], po3[:], evn[(oc, tt)][:])
                                nc.sync.dma_start(
                                    out_d.ap()[tt * 128:(tt + 1) * 128,
                                               oc * 512:(oc + 1) * 512],
                                    ob[:])

    nc.compile()
    return nc


def _maybe_install_trace_hook():
    try:
        import antenv
        from trn_agent_boot.trn_boot import _ntff_profile_via_ctypes
        hook = _ntff_profile_via_ctypes("/opt/axon/libaxon_pjrt.so")
        mod = types.ModuleType("antenv.axon_hooks")
        mod.get_axon_ntff_profile_hook = lambda: hook
        mod.set_axon_ntff_profile_hook = lambda h: None
        sys.modules["antenv.axon_hooks"] = mod
        antenv.axon_hooks = mod
        return True
    except Exception:
        return False


def kernel(x, w_attn, w_proj):
    x = np.ascontiguousarray(x, dtype=np.float32)
    w_attn = np.ascontiguousarray(w_attn, dtype=np.float32)
    w_proj = np.ascontiguousarray(w_proj, dtype=np.float32)

    if "nc" not in _cache:
        _cache["nc"] = _build()
    nc = _cache["nc"]

    xT = np.ascontiguousarray(x.T).astype(NP_BF16)
    wpT = np.ascontiguousarray(w_proj.T).astype(NP_BF16)
    in_maps = []
    for c in range(W):
        r0 = CL * c
        wqk = np.concatenate(
            [w_attn[r0:r0 + CL], w_attn[C + r0:C + r0 + CL]], axis=0)
        wqkT = np.ascontiguousarray(wqk.T).astype(NP_BF16)
        wvT = np.ascontiguousarray(
            w_attn[2 * C + r0:2 * C + r0 + CL].T).astype(NP_BF16)
        in_maps.append({"xT": xT, "wqkT": wqkT, "wvT": wvT, "wpT": wpT})

    trace = TRACE and _maybe_install_trace_hook()
    res = run_bass_kernel_spmd(nc, in_maps, list(range(W)), trace=trace)
    LAST_RESULT["exec_time_ns"] = res.exec_time_ns

    return np.concatenate([res.results[c]["out"] for c in range(W)], axis=0)
